# revision 13
# baseline (speedup 1.0000x reference)
"""Trainium2 Bass kernel for nn_DAC_structure (sparse dual-attention structure map).

For inputs q/k of shape (B*CH, L, H, E) = (64, 32, 8, 64):
  s  = softmax((q @ k^T) / sqrt(E))            per (batch-channel, head)
  m  = mean over the CH=8 channel group        -> [b, H, 32, 32]
  out_ps = element-repeat(m_ps, 32, 32)        -> [b, H, 1024, 1024]
  out_pn = tile(m_pn, 32, 32)                  -> [b, H, 1024, 1024]

Sharding: data-parallel over the true batch dim b = 8; core i handles batch i
(channel rows 8i..8i+8). No cross-device comms. Each core writes its own
[8, 1024, 1024] x2 output shard; the host stacks shards along axis 0.

The kernel is HBM-write-bound. The f32 baseline ran the output stream at
351 GB/s = the ~358 GB/s HBM-per-NeuronCore ceiling, so the only lever is
bytes: outputs are stored as bfloat16 (rel quantization error <= 2^-8
~ 0.4%, well inside the 2e-2 harness gate) and upcast to f32 on the host.
That halves the per-core write traffic from 64 MB to 32 MB.

Structure:
  Phase 1 (compute): load q/k (one 3-level-AP DMA per tensor), PE-transpose,
    64 QK^T matmuls per stream, batched softmax + channel-mean in f32, and
    cast into bf16 expansion source tiles in SBUF. All source tiles use the
    full 128 partitions — a 64-partition DMA source halves the usable SBUF
    ports and the write rate.
  Phase 2 (write): one back-to-back stream of output DMAs on the Sync ring:
      - out_ps: one 4 MB DMA per 4-head group; each source row re-read 32x
        via a stride-0 middle AP dim, dst walk fully sequential in HBM.
      - out_pn: per head, the 32-row tile block is partition-replicated 4x
        into a [128, 1024] bf16 tile (= 128 consecutive output rows) by
        Scalar-ring SBUF->SBUF DMAs during phase 1, then written as eight
        flat 256 KB DMAs.
"""

import sys

if "/opt/trn_rl_repo" not in sys.path:
    sys.path.insert(0, "/opt/trn_rl_repo")

from contextlib import ExitStack

import numpy as np

import concourse.bacc as bacc
import concourse.bass as bass
import concourse.mybir as mybir
import concourse.tile as tile
from concourse.masks import make_identity

F32 = mybir.dt.float32
F16 = mybir.dt.float16
BF16 = mybir.dt.bfloat16

CH = 8   # channels per true batch
L = 32   # patch_num (seq len of the small attention)
H = 8    # heads
E = 64   # head dim
WIN = 1024
N_CORES = 8


def _load_inputs(nc, pool, ins, kind, eng):
    """One 3-level-AP DMA per tensor into
    [128 = (c%4)*32 + l, 1024 = (c//4)*512 + h*64 + e].
    The ps pair rides the (otherwise idle until ~20us) Sync ring so the
    critical-path loads start as early as possible; the pn pair goes on
    Scalar so issue+transfer of the two pairs overlap."""
    nat = {}
    for name in ("q", "k"):
        dram = ins[name]
        t = pool.tile([128, 1024], F32, tag=f"nat_{kind}_{name}", name=f"nat_{kind}_{name}")
        src = bass.AP(tensor=dram.tensor, offset=dram.offset,
                      ap=[[H * E, 128], [4 * L * H * E, 2], [1, H * E]])
        dst = bass.AP(tensor=t.tensor, offset=t.offset,
                      ap=[list(t.ap[0]), [512, 2], [1, 512]])
        eng.dma_start(out=dst, in_=src)
        nat[name] = t
    return nat


def _transpose_group(nc, psum_t, ident, nat, tr, g):
    """PE-transpose the 128-col chunks needed by h-group g (h = 4g..4g+3)."""
    js = (0, 1, 4, 5) if g == 0 else (2, 3, 6, 7)
    for name in ("q", "k"):
        for j in js:
            pt = psum_t.tile([128, 128], F32, tag="ptrans", name="ptrans")
            nc.tensor.transpose(pt, nat[name][:, j * 128 : (j + 1) * 128], ident)
            nc.vector.tensor_copy(tr[name][:, j * 128 : (j + 1) * 128], pt)


def _group_mean_softmax(nc, pool, psum_s, tr, g, kind):
    """QK^T matmuls + softmax + channel mean for h-group g -> M [128, 32]."""
    s_ps = psum_s.tile([128, 256], F32, tag="spsum", name="spsum")
    for c in range(CH):
        chalf, clo = divmod(c, 4)
        for hh in range(4):
            h = g * 4 + hh
            col = (chalf * 4 + h // 2) * 128 + clo * 32
            prow = (h % 2) * 64
            nc.tensor.matmul(
                s_ps[hh * 32 : hh * 32 + 32, c * 32 : c * 32 + 32],
                tr["q"][prow : prow + 64, col : col + 32],
                tr["k"][prow : prow + 64, col : col + 32],
                start=True, stop=True,
                tile_position=(prow, hh * 32),
            )
    ex = pool.tile([128, 256], F32, tag=f"ex_{kind}", name=f"ex_{kind}")
    nc.scalar.activation(ex, s_ps, mybir.ActivationFunctionType.Exp, scale=1.0 / 8.0)
    r = pool.tile([128, 8], F32, tag=f"r_{kind}", name=f"r_{kind}")
    ex_cview = bass.AP(tensor=ex.tensor, offset=ex.offset,
                       ap=[list(ex.ap[0]), [32, 8], [1, 32]])
    nc.vector.tensor_reduce(r, ex_cview, axis=mybir.AxisListType.X, op=mybir.AluOpType.add)
    w = pool.tile([128, 8], F32, tag=f"w_{kind}", name=f"w_{kind}")
    nc.vector.reciprocal(w, r)
    wx = pool.tile([128, 256], F32, tag=f"wx_{kind}", name=f"wx_{kind}")
    ex_scl = bass.AP(tensor=ex.tensor, offset=ex.offset,
                     ap=[list(ex.ap[0]), [1, 32], [32, 8]])
    w_bc = bass.AP(tensor=w.tensor, offset=w.offset,
                   ap=[list(w.ap[0]), [0, 32], [1, 8]])
    wx_out = bass.AP(tensor=wx.tensor, offset=wx.offset,
                     ap=[list(wx.ap[0]), [8, 32], [1, 8]])
    nc.vector.scalar_tensor_tensor(out=wx_out, in0=ex_scl, scalar=1.0 / CH, in1=w_bc,
                                   op0=mybir.AluOpType.mult, op1=mybir.AluOpType.mult)
    m = pool.tile([128, 32], F32, tag=f"m_{kind}_{g}", name=f"m_{kind}_{g}")
    wx_in = bass.AP(tensor=wx.tensor, offset=wx.offset,
                    ap=[list(wx.ap[0]), [8, 32], [1, 8]])
    nc.vector.tensor_reduce(m, wx_in, axis=mybir.AxisListType.X, op=mybir.AluOpType.add)
    return m


def _build_ps_tile(nc, pool, m, g):
    """Element-repeat expansion tile [128, 4096] bf16 for h-group g: the
    expanded 2 KB row is duplicated 4x side by side so each write
    descriptor moves 8 KB contiguous->contiguous. (With a plain 2 KB
    stride-0 source, bf16 doubled the descriptor count per byte vs the
    f32 baseline, and SDMA engine 15 -- the known-slow descriptor path --
    degraded to 20.5 GB/s while the other 15 ran 23.3, adding a ~10 us
    straggler tail.)"""
    exp_t = pool.tile([128, 4096], BF16, tag=f"expand_ps_{g}", name=f"expand_ps_{g}")
    e1 = bass.AP(tensor=exp_t.tensor, offset=exp_t.offset,
                 ap=[list(exp_t.ap[0]), [32, 32], [1, 32]])
    src = bass.AP(tensor=m.tensor, offset=m.offset,
                  ap=[list(m.ap[0]), [1, 32], [0, 32]])
    nc.vector.tensor_copy(e1, src)
    dup_dst = bass.AP(tensor=exp_t.tensor, offset=exp_t.offset + 1024,
                      ap=[list(exp_t.ap[0]), [1024, 3], [1, 1024]])
    dup_src = bass.AP(tensor=exp_t.tensor, offset=exp_t.offset,
                      ap=[list(exp_t.ap[0]), [0, 3], [1, 1024]])
    nc.vector.tensor_copy(dup_dst, dup_src)
    return exp_t


def _build_block_identity(nc, pool, ident):
    """Wtile [128, 128] bf16 with Wtile[p, q] = 1 iff p%32 == q%32.
    Stationary operand for the pn partition-replication matmuls."""
    w = pool.tile([128, 128], BF16, tag="wrep", name="wrep")
    for hh in range(4):
        sl = ident[hh * 32 : (hh + 1) * 32, hh * 32 : (hh + 1) * 32]
        src = bass.AP(tensor=sl.tensor, offset=sl.offset,
                      ap=[list(sl.ap[0]), [0, 4], [1, 32]])
        dsl = w[hh * 32 : (hh + 1) * 32, :]
        dst = bass.AP(tensor=dsl.tensor, offset=dsl.offset,
                      ap=[list(dsl.ap[0]), [32, 4], [1, 32]])
        nc.vector.tensor_copy(dst, src)
    return w


def _build_pn_tiles(nc, pool, chunk_pool, psum_r, wrep, m, g):
    """Per-head partition-replication into [128, 1024] bf16 rep tiles
    (= 128 consecutive output rows each), entirely off the DMA path:
    a PE matmul with the block-identity replicates the head's 32 m-rows
    across all 128 partitions (PSUM [128, 32]), then DVE performs the
    32x tile-repeat during the PSUM->SBUF bf16 cast copy via a stride-0
    middle AP dim. (The f32 SBUF->SBUF DMA variant starved against the
    Sync-ring output stream: rep builds trickled at ~40 GB/s until 95us
    and the pn writes stalled on them.)"""
    m16 = chunk_pool.tile([128, 32], BF16, tag=f"m16_{g}", name=f"m16_{g}")
    nc.vector.tensor_copy(m16, m)
    reps = []
    for hh in range(4):
        h = g * 4 + hh
        msl = m16[hh * 32 : (hh + 1) * 32, :]
        pt = psum_r.tile([128, 32], F32, tag="prep", name="prep")
        nc.tensor.matmul(pt, wrep[hh * 32 : (hh + 1) * 32, :], msl,
                         start=True, stop=True, tile_position=(hh * 32, 0))
        rep = pool.tile([128, 1024], BF16, tag=f"rep_pn_{h}", name=f"rep_pn_{h}")
        src = bass.AP(tensor=pt.tensor, offset=pt.offset,
                      ap=[list(pt.ap[0]), [0, 32], [1, 32]])
        nc.vector.tensor_copy(rep, src)
        reps.append(rep)
    return reps


def _write_ps_group(nc, exp_t, out_dram, g):
    """One 4 MB DMA for h-group g: partition p holds the expanded row for
    (head hh = p//32, patch row l = p%32) duplicated 4x (8 KB) and is
    re-read 8x via a stride-0 middle AP dim to produce output rows
    32p..32p+32 in 8 KB descriptors. Partition p+1 continues exactly
    where p ended -> the HBM walk is fully sequential."""
    pitch = exp_t.ap[0][0]
    src = bass.AP(tensor=exp_t.tensor, offset=exp_t.offset,
                  ap=[[pitch, 128], [0, 8], [1, 4096]])
    dst = bass.AP(tensor=out_dram.tensor,
                  offset=out_dram.offset + g * 4096 * WIN,
                  ap=[[32 * WIN, 128], [4 * WIN, 8], [1, 4096]])
    nc.sync.dma_start(out=dst, in_=src)


def _write_pn_head(nc, rep, out_dram, h):
    """One 2 MB DMA for head h: the [128, 1024] rep tile (= 128 consecutive
    output rows) is re-read 8x via a stride-0 middle AP dim to fill all
    1024 rows. (Eight separate 256 KB flat DMAs were HWDGE-issue-bound:
    ~0.6 us issue vs ~0.7 us drain each left the ring starved and the pn
    phase ran at ~300 GB/s with dips to 174.)"""
    pitch = rep.ap[0][0]
    src = bass.AP(tensor=rep.tensor, offset=rep.offset,
                  ap=[[pitch, 128], [0, 8], [1, 1024]])
    dst = bass.AP(tensor=out_dram.tensor,
                  offset=out_dram.offset + h * WIN * WIN,
                  ap=[[WIN, 128], [128 * WIN, 8], [1, 1024]])
    nc.sync.dma_start(out=dst, in_=src)


def build_program():
    """Build and compile the per-core Bass program. Returns the Bacc object."""
    nc = bacc.Bacc(
        "TRN2",
        target_bir_lowering=False,
        debug=False,
        enable_asserts=False,
        num_devices=N_CORES,
    )
    ins = {}
    for name in ("qps", "qpn", "kps", "kpn"):
        ins[name] = nc.dram_tensor(name, [CH, L, H, E], F32, kind="ExternalInput").ap()
    out_ps = nc.dram_tensor("out_ps", [H, WIN, WIN], BF16, kind="ExternalOutput").ap()
    out_pn = nc.dram_tensor("out_pn", [H, WIN, WIN], BF16, kind="ExternalOutput").ap()

    with tile.TileContext(nc) as tc:
        with ExitStack() as ctx:
            pool = ctx.enter_context(tc.tile_pool(name="sbuf", bufs=1))
            chunk_pool = ctx.enter_context(tc.tile_pool(name="chunks", bufs=2))
            psum_t = ctx.enter_context(tc.tile_pool(name="ptrans", bufs=3, space="PSUM"))
            psum_s = ctx.enter_context(tc.tile_pool(name="spsum", bufs=2, space="PSUM"))
            psum_r = ctx.enter_context(tc.tile_pool(name="prep", bufs=2, space="PSUM"))
            ident = pool.tile([128, 128], F32, tag="ident")
            make_identity(nc, ident)
            wrep = _build_block_identity(nc, pool, ident)

            nat_ps = _load_inputs(nc, pool, {"q": ins["qps"], "k": ins["kps"]}, "ps",
                                  nc.sync)
            nat_pn = _load_inputs(nc, pool, {"q": ins["qpn"], "k": ins["kpn"]}, "pn",
                                  nc.scalar)
            # fp16 transposed tiles: halves the PSUM->SBUF copy bytes and
            # doubles QK matmul rate on the PE; fp16 q/k quantization adds
            # only ~0.4% total rel err (host-verified), far inside the gate.
            tr_ps = {n: pool.tile([128, 1024], F16, tag=f"tr_ps_{n}",
                                  name=f"tr_ps_{n}") for n in ("q", "k")}
            tr_pn = {n: pool.tile([128, 1024], F16, tag=f"tr_pn_{n}",
                                  name=f"tr_pn_{n}") for n in ("q", "k")}

            # Phase 1: compute everything into persistent SBUF tiles
            ps_tiles, pn_reps = [], []
            for g in range(2):
                _transpose_group(nc, psum_t, ident, nat_ps, tr_ps, g)
                m = _group_mean_softmax(nc, chunk_pool, psum_s, tr_ps, g, "ps")
                ps_tiles.append(_build_ps_tile(nc, pool, m, g))

                _transpose_group(nc, psum_t, ident, nat_pn, tr_pn, g)
                m = _group_mean_softmax(nc, chunk_pool, psum_s, tr_pn, g, "pn")
                pn_reps.extend(_build_pn_tiles(nc, pool, chunk_pool, psum_r, wrep, m, g))

            # Phase 2: one back-to-back output write stream on the Sync ring.
            # ps g0 (ready first) leads; the strided-walk pn heads run
            # mid-stream where overlapping DMAs hide their latency; the
            # fully-sequential ps g1 write goes LAST so the solo tail
            # drains at full rate (a strided pn DMA alone at the tail
            # trickled at ~115 GB/s and cost ~8 us).
            _write_ps_group(nc, ps_tiles[0], out_ps, 0)
            for h in range(H):
                _write_pn_head(nc, pn_reps[h], out_pn, h)
            _write_ps_group(nc, ps_tiles[1], out_ps, 1)
    nc.compile()
    return nc


_NC_CACHE = None


def _get_nc():
    global _NC_CACHE
    if _NC_CACHE is None:
        _NC_CACHE = build_program()
    return _NC_CACHE


def run_sharded(queries_patch_size, queries_patch_num, keys_patch_size, keys_patch_num,
                trace=False, tmpdir=None):
    """Run the SPMD kernel on 8 cores; returns (full_ps, full_pn[, results])."""
    from concourse.bass_utils import run_bass_kernel_spmd

    nc = _get_nc()
    qps = np.ascontiguousarray(np.asarray(queries_patch_size, dtype=np.float32))
    qpn = np.ascontiguousarray(np.asarray(queries_patch_num, dtype=np.float32))
    kps = np.ascontiguousarray(np.asarray(keys_patch_size, dtype=np.float32))
    kpn = np.ascontiguousarray(np.asarray(keys_patch_num, dtype=np.float32))

    in_maps = []
    for b in range(N_CORES):
        sl = slice(b * CH, (b + 1) * CH)
        in_maps.append({
            "qps": qps[sl], "qpn": qpn[sl], "kps": kps[sl], "kpn": kpn[sl],
        })
    res = run_bass_kernel_spmd(nc, in_maps, core_ids=list(range(N_CORES)), trace=trace,
                               tmpdir=tmpdir)
    full_ps = np.stack(
        [np.asarray(res.results[b]["out_ps"]).astype(np.float32) for b in range(N_CORES)],
        axis=0)
    full_pn = np.stack(
        [np.asarray(res.results[b]["out_pn"]).astype(np.float32) for b in range(N_CORES)],
        axis=0)
    if trace:
        return full_ps, full_pn, res
    return full_ps, full_pn


def kernel(queries_patch_size, queries_patch_num, keys_patch_size, keys_patch_num,
           values=None, patch_index=0, attn_mask=None):
    """Full-input entry point: takes the unsharded inputs, returns full outputs."""
    full_ps, full_pn = run_sharded(
        queries_patch_size, queries_patch_num, keys_patch_size, keys_patch_num
    )
    return full_ps, full_pn


# revision 17
# speedup vs baseline: 1.1642x; 1.1642x over previous
"""Trainium2 Bass kernel for nn_DAC_structure (sparse dual-attention structure map).

For inputs q/k of shape (B*CH, L, H, E) = (64, 32, 8, 64):
  s  = softmax((q @ k^T) / sqrt(E))            per (batch-channel, head)
  m  = mean over the CH=8 channel group        -> [b, H, 32, 32]
  out_ps = element-repeat(m_ps, 32, 32)        -> [b, H, 1024, 1024]
  out_pn = tile(m_pn, 32, 32)                  -> [b, H, 1024, 1024]

Sharding: data-parallel over the true batch dim b = 8; core i handles batch i
(channel rows 8i..8i+8). No cross-device comms. Each core writes its own
[8, 1024, 1024] x2 output shard; the host stacks shards along axis 0.

The kernel is HBM-write-bound. Measured facts driving the design:
  * Outputs are stored bf16 (quantization ~0.4% rel err, harness gate is
    2e-2) and upcast to f32 on the host: halves write traffic to
    32 MB/core. The f32 stream ran at the ~358 GB/s HBM-per-NC ceiling,
    so bytes were the only lever.
  * All output DMAs use the full 128 partitions: DMAs on partition
    sub-ranges get catastrophically imbalanced engine splits (a 92/28/4
    partition-split variant put 85% of bytes on 4 of 16 SDMA engines and
    ran at ~100 GB/s).
  * 8 KB descriptors (4x-duplicated ps source rows, contiguous->
    contiguous) run 409-423 GB/s vs ~380 at 2 KB.
  * pn rep tiles are built on the PE (replication matmul with a block
    identity) + DVE stride-0 PSUM->SBUF cast copies: SBUF->SBUF DMA
    builds starved against the write stream (~40 GB/s) and stalled the
    pn writes behind them.
  * Per-head pn writes are single 2 MB DMAs (stride-0 source re-read):
    64 x 256 KB flat DMAs were HWDGE-issue-bound (~0.6 us issue vs
    ~0.7 us drain).
  * The fully-sequential ps g1 write goes last: a strided pn DMA
    draining solo at the tail runs at ~115 GB/s.
  * q/k loads ride different rings (Sync/Scalar) so they transfer
    concurrently; serializing them delayed the k-transposes ~4 us.
"""

import sys

if "/opt/trn_rl_repo" not in sys.path:
    sys.path.insert(0, "/opt/trn_rl_repo")

from contextlib import ExitStack

import numpy as np

import concourse.bacc as bacc
import concourse.bass as bass
import concourse.mybir as mybir
import concourse.tile as tile
from concourse.masks import make_identity

F32 = mybir.dt.float32
F16 = mybir.dt.float16
BF16 = mybir.dt.bfloat16

CH = 8   # channels per true batch
L = 32   # patch_num (seq len of the small attention)
H = 8    # heads
E = 64   # head dim
WIN = 1024
N_CORES = 8


def _load_inputs(nc, pool, ins, kind):
    """One 3-level-AP DMA per tensor into
    [128 = (c%4)*32 + l, 1024 = (c//4)*512 + h*64 + e]; q on Sync,
    k on Scalar so the pair transfers concurrently."""
    nat = {}
    for name, eng in (("q", nc.sync), ("k", nc.scalar)):
        dram = ins[name]
        t = pool.tile([128, 1024], F32, tag=f"nat_{kind}_{name}", name=f"nat_{kind}_{name}")
        src = bass.AP(tensor=dram.tensor, offset=dram.offset,
                      ap=[[H * E, 128], [4 * L * H * E, 2], [1, H * E]])
        dst = bass.AP(tensor=t.tensor, offset=t.offset,
                      ap=[list(t.ap[0]), [512, 2], [1, 512]])
        eng.dma_start(out=dst, in_=src)
        nat[name] = t
    return nat


def _transpose_group(nc, psum_t, ident, nat, tr, g):
    """PE-transpose the 128-col chunks needed by h-group g (h = 4g..4g+3)."""
    js = (0, 1, 4, 5) if g == 0 else (2, 3, 6, 7)
    for name in ("q", "k"):
        for j in js:
            pt = psum_t.tile([128, 128], F32, tag="ptrans", name="ptrans")
            nc.tensor.transpose(pt, nat[name][:, j * 128 : (j + 1) * 128], ident)
            nc.vector.tensor_copy(tr[name][:, j * 128 : (j + 1) * 128], pt)


def _group_mean_softmax(nc, pool, psum_s, tr, g, kind):
    """QK^T matmuls + softmax + channel mean for h-group g -> M [128, 32]."""
    s_ps = psum_s.tile([128, 256], F32, tag="spsum", name="spsum")
    for c in range(CH):
        chalf, clo = divmod(c, 4)
        for hh in range(4):
            h = g * 4 + hh
            col = (chalf * 4 + h // 2) * 128 + clo * 32
            prow = (h % 2) * 64
            nc.tensor.matmul(
                s_ps[hh * 32 : hh * 32 + 32, c * 32 : c * 32 + 32],
                tr["q"][prow : prow + 64, col : col + 32],
                tr["k"][prow : prow + 64, col : col + 32],
                start=True, stop=True,
                tile_position=(prow, hh * 32),
            )
    ex = pool.tile([128, 256], F32, tag=f"ex_{kind}", name=f"ex_{kind}")
    nc.scalar.activation(ex, s_ps, mybir.ActivationFunctionType.Exp, scale=1.0 / 8.0)
    r = pool.tile([128, 8], F32, tag=f"r_{kind}", name=f"r_{kind}")
    ex_cview = bass.AP(tensor=ex.tensor, offset=ex.offset,
                       ap=[list(ex.ap[0]), [32, 8], [1, 32]])
    nc.vector.tensor_reduce(r, ex_cview, axis=mybir.AxisListType.X, op=mybir.AluOpType.add)
    w = pool.tile([128, 8], F32, tag=f"w_{kind}", name=f"w_{kind}")
    nc.vector.reciprocal(w, r)
    wx = pool.tile([128, 256], F32, tag=f"wx_{kind}", name=f"wx_{kind}")
    ex_scl = bass.AP(tensor=ex.tensor, offset=ex.offset,
                     ap=[list(ex.ap[0]), [1, 32], [32, 8]])
    w_bc = bass.AP(tensor=w.tensor, offset=w.offset,
                   ap=[list(w.ap[0]), [0, 32], [1, 8]])
    wx_out = bass.AP(tensor=wx.tensor, offset=wx.offset,
                     ap=[list(wx.ap[0]), [8, 32], [1, 8]])
    nc.vector.scalar_tensor_tensor(out=wx_out, in0=ex_scl, scalar=1.0 / CH, in1=w_bc,
                                   op0=mybir.AluOpType.mult, op1=mybir.AluOpType.mult)
    m = pool.tile([128, 32], F32, tag=f"m_{kind}_{g}", name=f"m_{kind}_{g}")
    wx_in = bass.AP(tensor=wx.tensor, offset=wx.offset,
                    ap=[list(wx.ap[0]), [8, 32], [1, 8]])
    nc.vector.tensor_reduce(m, wx_in, axis=mybir.AxisListType.X, op=mybir.AluOpType.add)
    return m


def _build_ps_tile(nc, pool, m, g):
    """Element-repeat expansion tile [128, 4096] bf16 for h-group g: the
    expanded 2 KB row is duplicated 4x side by side so each write
    descriptor moves 8 KB contiguous->contiguous."""
    exp_t = pool.tile([128, 4096], BF16, tag=f"expand_ps_{g}", name=f"expand_ps_{g}")
    e1 = bass.AP(tensor=exp_t.tensor, offset=exp_t.offset,
                 ap=[list(exp_t.ap[0]), [32, 32], [1, 32]])
    src = bass.AP(tensor=m.tensor, offset=m.offset,
                  ap=[list(m.ap[0]), [1, 32], [0, 32]])
    nc.vector.tensor_copy(e1, src)
    dup_dst = bass.AP(tensor=exp_t.tensor, offset=exp_t.offset + 1024,
                      ap=[list(exp_t.ap[0]), [1024, 3], [1, 1024]])
    dup_src = bass.AP(tensor=exp_t.tensor, offset=exp_t.offset,
                      ap=[list(exp_t.ap[0]), [0, 3], [1, 1024]])
    nc.vector.tensor_copy(dup_dst, dup_src)
    return exp_t


def _build_block_identity(nc, pool, ident):
    """Wtile [128, 128] bf16 with Wtile[p, q] = 1 iff p%32 == q%32.
    Stationary operand for the pn partition-replication matmuls."""
    w = pool.tile([128, 128], BF16, tag="wrep", name="wrep")
    for hh in range(4):
        sl = ident[hh * 32 : (hh + 1) * 32, hh * 32 : (hh + 1) * 32]
        src = bass.AP(tensor=sl.tensor, offset=sl.offset,
                      ap=[list(sl.ap[0]), [0, 4], [1, 32]])
        dsl = w[hh * 32 : (hh + 1) * 32, :]
        dst = bass.AP(tensor=dsl.tensor, offset=dsl.offset,
                      ap=[list(dsl.ap[0]), [32, 4], [1, 32]])
        nc.vector.tensor_copy(dst, src)
    return w


def _build_pn_tiles(nc, pool, chunk_pool, psum_r, wrep, m, g):
    """Per-head partition-replication into [128, 1024] bf16 rep tiles
    (= 128 consecutive output rows each), entirely off the DMA path:
    a PE matmul with the block-identity replicates the head's 32 m-rows
    across all 128 partitions (PSUM [128, 32]), then DVE performs the
    32x tile-repeat during the PSUM->SBUF bf16 cast copy via a stride-0
    middle AP dim."""
    m16 = chunk_pool.tile([128, 32], BF16, tag=f"m16_{g}", name=f"m16_{g}")
    nc.vector.tensor_copy(m16, m)
    reps = []
    for hh in range(4):
        h = g * 4 + hh
        msl = m16[hh * 32 : (hh + 1) * 32, :]
        pt = psum_r.tile([128, 32], F32, tag="prep", name="prep")
        nc.tensor.matmul(pt, wrep[hh * 32 : (hh + 1) * 32, :], msl,
                         start=True, stop=True, tile_position=(hh * 32, 0))
        rep = pool.tile([128, 1024], BF16, tag=f"rep_pn_{h}", name=f"rep_pn_{h}")
        src = bass.AP(tensor=pt.tensor, offset=pt.offset,
                      ap=[list(pt.ap[0]), [0, 32], [1, 32]])
        nc.vector.tensor_copy(rep, src)
        reps.append(rep)
    return reps


def _write_ps_group(nc, exp_t, out_dram, g):
    """One 8 MB DMA for h-group g: partition p holds the expanded row for
    (head hh = p//32, patch row l = p%32) duplicated 4x (8 KB) and is
    re-read 8x via a stride-0 middle AP dim to produce output rows
    32p..32p+32 in 8 KB descriptors. Partition p+1 continues exactly
    where p ended -> the HBM walk is fully sequential."""
    pitch = exp_t.ap[0][0]
    src = bass.AP(tensor=exp_t.tensor, offset=exp_t.offset,
                  ap=[[pitch, 128], [0, 8], [1, 4096]])
    dst = bass.AP(tensor=out_dram.tensor,
                  offset=out_dram.offset + g * 4096 * WIN,
                  ap=[[32 * WIN, 128], [4 * WIN, 8], [1, 4096]])
    nc.sync.dma_start(out=dst, in_=src)


def _write_pn_head(nc, rep, out_dram, h):
    """One 2 MB DMA for head h: the [128, 1024] rep tile (= 128
    consecutive output rows) re-read 8x via a stride-0 middle AP dim to
    fill all 1024 rows."""
    pitch = rep.ap[0][0]
    src = bass.AP(tensor=rep.tensor, offset=rep.offset,
                  ap=[[pitch, 128], [0, 8], [1, 1024]])
    dst = bass.AP(tensor=out_dram.tensor,
                  offset=out_dram.offset + h * WIN * WIN,
                  ap=[[WIN, 128], [128 * WIN, 8], [1, 1024]])
    nc.sync.dma_start(out=dst, in_=src)


def build_program():
    """Build and compile the per-core Bass program. Returns the Bacc object."""
    nc = bacc.Bacc(
        "TRN2",
        target_bir_lowering=False,
        debug=False,
        enable_asserts=False,
        num_devices=N_CORES,
    )
    ins = {}
    for name in ("qps", "qpn", "kps", "kpn"):
        ins[name] = nc.dram_tensor(name, [CH, L, H, E], F32, kind="ExternalInput").ap()
    out_ps = nc.dram_tensor("out_ps", [H, WIN, WIN], BF16, kind="ExternalOutput").ap()
    out_pn = nc.dram_tensor("out_pn", [H, WIN, WIN], BF16, kind="ExternalOutput").ap()

    with tile.TileContext(nc) as tc:
        with ExitStack() as ctx:
            pool = ctx.enter_context(tc.tile_pool(name="sbuf", bufs=1))
            chunk_pool = ctx.enter_context(tc.tile_pool(name="chunks", bufs=2))
            psum_t = ctx.enter_context(tc.tile_pool(name="ptrans", bufs=3, space="PSUM"))
            psum_s = ctx.enter_context(tc.tile_pool(name="spsum", bufs=2, space="PSUM"))
            psum_r = ctx.enter_context(tc.tile_pool(name="prep", bufs=2, space="PSUM"))
            ident = pool.tile([128, 128], F32, tag="ident")
            make_identity(nc, ident)
            wrep = _build_block_identity(nc, pool, ident)

            nat_ps = _load_inputs(nc, pool, {"q": ins["qps"], "k": ins["kps"]}, "ps")
            nat_pn = _load_inputs(nc, pool, {"q": ins["qpn"], "k": ins["kpn"]}, "pn")
            # fp16 transposed tiles: halves the PSUM->SBUF copy bytes and
            # doubles QK matmul rate; fp16 q/k quantization adds ~0.4%
            # total rel err (host-verified), far inside the 2e-2 gate.
            tr_ps = {n: pool.tile([128, 1024], F16, tag=f"tr_ps_{n}",
                                  name=f"tr_ps_{n}") for n in ("q", "k")}
            tr_pn = {n: pool.tile([128, 1024], F16, tag=f"tr_pn_{n}",
                                  name=f"tr_pn_{n}") for n in ("q", "k")}

            # Phase 1: compute everything into persistent SBUF tiles
            ps_tiles, pn_reps = [], []
            for g in range(2):
                _transpose_group(nc, psum_t, ident, nat_ps, tr_ps, g)
                m = _group_mean_softmax(nc, chunk_pool, psum_s, tr_ps, g, "ps")
                ps_tiles.append(_build_ps_tile(nc, pool, m, g))

                _transpose_group(nc, psum_t, ident, nat_pn, tr_pn, g)
                m = _group_mean_softmax(nc, chunk_pool, psum_s, tr_pn, g, "pn")
                pn_reps.extend(_build_pn_tiles(nc, pool, chunk_pool, psum_r, wrep, m, g))

            # Phase 2: one back-to-back output write stream on the Sync
            # ring. ps g0 (ready first) leads; the strided-walk pn heads
            # run mid-stream where overlapping DMAs hide their latency;
            # the fully-sequential ps g1 goes last so the solo tail
            # drains at full rate.
            _write_ps_group(nc, ps_tiles[0], out_ps, 0)
            for h in range(H):
                _write_pn_head(nc, pn_reps[h], out_pn, h)
            _write_ps_group(nc, ps_tiles[1], out_ps, 1)
    nc.compile()
    return nc


_NC_CACHE = None


def _get_nc():
    global _NC_CACHE
    if _NC_CACHE is None:
        _NC_CACHE = build_program()
    return _NC_CACHE


def run_sharded(queries_patch_size, queries_patch_num, keys_patch_size, keys_patch_num,
                trace=False, tmpdir=None):
    """Run the SPMD kernel on 8 cores; returns (full_ps, full_pn[, results])."""
    from concourse.bass_utils import run_bass_kernel_spmd

    nc = _get_nc()
    qps = np.ascontiguousarray(np.asarray(queries_patch_size, dtype=np.float32))
    qpn = np.ascontiguousarray(np.asarray(queries_patch_num, dtype=np.float32))
    kps = np.ascontiguousarray(np.asarray(keys_patch_size, dtype=np.float32))
    kpn = np.ascontiguousarray(np.asarray(keys_patch_num, dtype=np.float32))

    in_maps = []
    for b in range(N_CORES):
        sl = slice(b * CH, (b + 1) * CH)
        in_maps.append({
            "qps": qps[sl], "qpn": qpn[sl], "kps": kps[sl], "kpn": kpn[sl],
        })
    res = run_bass_kernel_spmd(nc, in_maps, core_ids=list(range(N_CORES)), trace=trace,
                               tmpdir=tmpdir)
    full_ps = np.stack(
        [np.asarray(res.results[b]["out_ps"]).astype(np.float32) for b in range(N_CORES)],
        axis=0)
    full_pn = np.stack(
        [np.asarray(res.results[b]["out_pn"]).astype(np.float32) for b in range(N_CORES)],
        axis=0)
    if trace:
        return full_ps, full_pn, res
    return full_ps, full_pn


def kernel(queries_patch_size, queries_patch_num, keys_patch_size, keys_patch_num,
           values=None, patch_index=0, attn_mask=None):
    """Full-input entry point: takes the unsharded inputs, returns full outputs."""
    full_ps, full_pn = run_sharded(
        queries_patch_size, queries_patch_num, keys_patch_size, keys_patch_num
    )
    return full_ps, full_pn
